# revision 67
# baseline (speedup 1.0000x reference)
"""MinLSTM cell (Heinsen-scan reference) as a Bass/Tile kernel for 8 trn2 NeuronCores.

The reference's log-space parallel scan is rewritten exactly in linear space:
    h_t = f'_t h_{t-1} + (1 - f'_t) g(pre_h_t),   h_0 = 1e-6
with f' = sigmoid(pre_f+b_f) / (sigmoid(pre_f+b_f) + sigmoid(pre_i+b_i)) and
g(x) = x>=0 ? x+0.5 : sigmoid(x). This is a convex combination of positive
terms, so it is numerically stable without log-space.

Distribution: data-parallel over batch N=8, one batch element per core, W/b
replicated. Host-side prep transposes x[n] to [H_in, L] and ships it TWICE:
e4m3 for the F/I gate matmuls (DoubleRow perf mode = 2 k-blocks per matmul
at ~2x PE rate; the f/(f+i) normalization damps fp8 error to ~8e-3) and fp16
for the H gate (its error hits the output with slope 1 — fp8 there alone
costs 1.8e-2 rel err, over the gate). W rows are regrouped per 128-channel
c-tile. Device output is [H, L] fp16, transposed/upcast on the host.

Per-core device pipeline (chunks round-robin over the 4 c-tiles, sizes
[512, 1024 x3, 512] per c so all four scan chains start early and drain
concurrently at the end):
  PE : psF/psI via fp8 DoubleRow matmuls; psH via fp16 matmuls (fp32 acc)
  ACT: sigf = sigmoid(psF+b_f); sigi = sigmoid(psI+b_i)   (fp16 out)
       shlh = sigmoid(psH+b_h); rl = psH + (b_h+0.5)      (Identity w/ bias)
  DVE: fp   = FP_FUSED_MINLSTM(sigf, sigi)   custom op: sf * recip1nr(sf+si)
              (bit-trick recip seed + one NR, 7 uop stages, ~0.2% max err)
       fpm1 = fp - 1                         (tensor_scalar, 4x fp16)
       htil = max(rl, shlh)  == g, exact     (tensor_tensor, 2x fp16)
       wv   = fpm1 * htil                    (tensor_tensor, 2x fp16)
       h    = tensor_tensor_scan(d0=fp, d1=wv, op0=mult, op1=subtract)
              => h_t = f'_t h_{t-1} - wv_t, chained across chunks via initial=
  DMA: h chunk -> HBM (sync queue; x16 also sync, x8 scalar — HWDGE only)
The DVE is the bottleneck engine (~84us busy: the 2 cyc/elem scan plus the
1x custom op; the rest run 2x/4x); PE ~66us, ACT ~74us, all overlapped by
the Tile scheduler. Startup floor is ~12us: engine preamble + the one-time
sigmoid act-table load gate the first real ACT op.
"""

import os
import sys

import numpy as np

sys.path.insert(0, "/opt/trn_rl_repo")

import concourse.bass as bass  # noqa: E402
import concourse.tile as tile  # noqa: E402
from concourse import bacc, mybir  # noqa: E402
import concourse.dve_ops as dve_ops_mod  # noqa: E402
from concourse.dve_spec import (  # noqa: E402
    C0,
    C1,
    AluOp,
    Bin,
    Spec,
    Src0,
    Src1,
    maxx,
)

RECIP_C0 = -0.23549792
RECIP_C1 = 2.0017324


def _make_dve_ops():
    """Register two kernel-specific fused DVE ops (idempotent).

    FP_FUSED_MINLSTM: out = Src0 * recip_1nr(Src0 + Src1) — the gate
    normalization f' = sf/(sf+si) in ONE 7-stage instruction (bit-trick
    reciprocal seed + one NR pass, ~0.2% max err; a second NR would not fit
    the 8-stage budget). Kills the CCE accumulate, the separate recip op and
    the f' multiply, and lets sigi be fp16.

    HTIL_MAX_MINLSTM: out = max(Src0 + C0, Src1) — g(x) = max(x+b+0.5,
    sigmoid(x+b)) with the per-partition bias riding C0, reading pre_h
    straight from PSUM. Kills the ACT Identity pass.
    """
    if "FP_FUSED_MINLSTM" in dve_ops_mod._SUB_OPCODE_FOR_NAME:
        by_name = {op.name: op for op in dve_ops_mod.OPS}
        return by_name["FP_FUSED_MINLSTM"], by_name["HTIL_MAX_MINLSTM"]

    _s = Bin(AluOp.ADD, Src0, Src1)
    _ns = Bin(AluOp.BITWISE_NOT, _s, _s)
    _y0 = _ns * C0
    _y1 = _y0 * (C1 - _s * _y0)

    def _ref_fp(in0, in1, c0, c1, c2):
        s = in0.astype(np.float32) + in1.astype(np.float32)
        not_x = (~s.view(np.int32)).view(np.float32)
        y0 = not_x * c0
        y1 = y0 * (c1 - s * y0)
        return y1 * in0.astype(np.float32)

    fp_op = dve_ops_mod.DveOp(
        "FP_FUSED_MINLSTM",
        Spec(body=_y1 * Src0, reference=_ref_fp),
        subdim=False,
        uops_sha={},
    )

    def _ref_htil(in0, in1, c0, c1, c2):
        return np.maximum(in0.astype(np.float32) + c0, in1.astype(np.float32))

    htil_op = dve_ops_mod.DveOp(
        "HTIL_MAX_MINLSTM",
        Spec(body=maxx(Src0 + C0, Src1), reference=_ref_htil),
        subdim=False,
        uops_sha={},
    )

    import re

    for op in (fp_op, htil_op):
        dve_ops_mod.OPS.append(op)
        dve_ops_mod._SUB_OPCODE_FOR_NAME[op.name] = (
            dve_ops_mod._CUSTOM_DVE_ROW_BASE + len(dve_ops_mod.OPS) - 1
        )
        dve_ops_mod.CUSTOM_DVE_SPECS[op.name] = op.spec
        for ver in ("v3", "v4"):
            try:
                op.compile(ver)
            except ValueError as e:
                m = re.search(r'="([0-9a-f]+)"', str(e))
                if not m:
                    raise
                op.uops_sha[ver] = m.group(1)
                op.compile(ver)
    return fp_op, htil_op

N, L, H_IN, H = 8, 4096, 512, 512
H3 = 3 * H
P = 128
NK = H_IN // P  # 4 k-blocks of the contraction dim
NCT = H // P  # 4 channel tiles
LT = 512  # psum/matmul l-tile (one PSUM bank of fp32)
LH = 2048  # l-granularity of the big DVE ops
NLT = L // LT
NLH = L // LH

F32 = mybir.dt.float32
F16 = mybir.dt.float16
F8 = mybir.dt.float8e4
Alu = mybir.AluOpType
Act = mybir.ActivationFunctionType
DoubleRow = mybir.MatmulPerfMode.DoubleRow

HX_INIT = 1e-6

_cached_nc = {}


def build_program(L=L, LH=LH):
    key = (L, LH)
    if key in _cached_nc:
        return _cached_nc[key]
    NLH = L // LH
    FP_OP, HTIL_OP = _make_dve_ops()

    nc = bacc.Bacc()
    # F/I gates run e4m3 DoubleRow matmuls (2x PE rate, gate error is damped
    # by the f/(f+i) normalization); the H gate runs fp16 (its error hits the
    # output with slope ~1, fp8 there alone costs 1.8e-2 rel err).
    xT8_d = nc.dram_tensor("xT8", [H_IN, L], F8, kind="ExternalInput")
    xT_d = nc.dram_tensor("xT", [H_IN, L], F16, kind="ExternalInput")
    wT8_d = nc.dram_tensor("wT8", [H_IN, 2 * H], F8, kind="ExternalInput")
    wT_d = nc.dram_tensor("wT", [H_IN, H], F16, kind="ExternalInput")
    bias_d = nc.dram_tensor("bias", [P, 16], F32, kind="ExternalInput")
    out_d = nc.dram_tensor("out", [H, L], F16, kind="ExternalOutput")

    with tile.TileContext(nc) as tc:
        with (
            tc.tile_pool(name="const", bufs=1) as const_pool,
            tc.tile_pool(name="gates", bufs=3) as gates_pool,
            tc.tile_pool(name="sig3", bufs=6) as sig3_pool,
            tc.tile_pool(name="scanbuf", bufs=4) as scan_pool,
            tc.tile_pool(name="psum", bufs=4, space="PSUM") as psum_pool,
        ):
            # Warmup activation with minimal sync deps: absorbs the one-time
            # sigmoid act-table load (walrus rejects table-load + multi-wait
            # on one Activation instruction).
            warm = const_pool.tile([P, 8], F32)
            nc.vector.memset(warm[:], 0.0)
            neg1 = const_pool.tile([P, 1], F32)
            nc.vector.memset(neg1[:], -1.0)
            # bias= so the instruction form matches the real gate sigmoids
            # (a formally different activation triggers a second 1.3us
            # LoadActFuncSet at startup).
            nc.scalar.activation(
                warm[:], warm[:], Act.Sigmoid, bias=warm[:, 0:1]
            )
            # PE warmup: ~2us of garbage matmuls with no dependencies, so the
            # HAM clock gate ramps while the first DMAs are in flight. (Kept
            # short: an in-order PE queue means every extra warmup matmul
            # delays the first real one.)
            wup = const_pool.tile([P, P], F16)
            nc.vector.memset(wup[:], 0.0)
            wup_ps = psum_pool.tile([P, P], F32, tag="ps")
            # Enough garbage matmuls to keep the PE clock from dropping back
            # to the LOW pstate while the first W/x DMAs land (~2.3us idle
            # otherwise; idle early matmuls run 4x slow at 0.65GHz).
            for _ in range(12):
                nc.tensor.matmul(wup_ps[:], wup[:], wup[:], start=True, stop=True)

            xT8_sb = const_pool.tile([P, NK, L], F8)
            xT_sb = const_pool.tile([P, NK, L], F16)
            wT8_sb = const_pool.tile([P, NK, 2 * H], F8)
            wT_sb = const_pool.tile([P, NK, H], F16)
            bias_sb = const_pool.tile([P, 16], F32)

            # W columns are host-reordered grouped by c-tile: wT8 holds each
            # c-tile's F/I blocks contiguous, wT (fp16) the H blocks; the
            # first DMA of each unblocks c=0. bias rides the sync queue.
            wT8_r = wT8_d.rearrange("(ki p) o -> p ki o", p=P)
            wT_r = wT_d.rearrange("(ki p) o -> p ki o", p=P)
            xT8_r = xT8_d.rearrange("(ki p) l -> p ki l", p=P)
            xT_r = xT_d.rearrange("(ki p) l -> p ki l", p=P)
            if L >= 4096:
                xchunks = [512] + [2048] * ((L - 2048) // 2048) + [1536]
            else:
                xchunks = [512] * (L // 512)
            CW = 2 * P

            # Sync-queue order: c0 weights + bias, FIRST x16 chunk (so round
            # 0 unblocks asap), then the remaining weight groups, then the
            # rest of x16. x8 rides the scalar queue (both HWDGE; the gpsimd
            # SWDGE queue generates descriptors in software — far too slow).
            def w_group(cg):
                nc.sync.dma_start(
                    wT8_sb[:, :, cg * CW : (cg + 1) * CW],
                    wT8_r[:, :, cg * CW : (cg + 1) * CW],
                )
                nc.sync.dma_start(
                    wT_sb[:, :, cg * P : (cg + 1) * P],
                    wT_r[:, :, cg * P : (cg + 1) * P],
                )

            # Head DMAs all on the sync queue; they land serially (~1.3-2us
            # each), and the critical path is psF <- F-matmuls <- {W8c0,
            # x8#1}, so those two lead. Alternatives measured WORSE: scalar
            # queue stalls the activation stream (+19us); gpsimd SWDGE has a
            # ~8us descriptor stall (+4us).
            nc.sync.dma_start(
                wT8_sb[:, :, :CW], wT8_r[:, :, :CW]
            )
            nc.sync.dma_start(
                xT8_sb[:, :, : xchunks[0]], xT8_r[:, :, : xchunks[0]]
            )
            nc.sync.dma_start(bias_sb[:], bias_d[:])
            nc.sync.dma_start(
                xT_sb[:, :, : xchunks[0]], xT_r[:, :, : xchunks[0]]
            )
            nc.sync.dma_start(
                wT_sb[:, :, :P], wT_r[:, :, :P]
            )
            for cg in range(1, NCT):
                w_group(cg)
            xoff = xchunks[0]
            for xch in xchunks[1:]:
                nc.sync.dma_start(
                    xT8_sb[:, :, xoff : xoff + xch],
                    xT8_r[:, :, xoff : xoff + xch],
                )
                nc.sync.dma_start(
                    xT_sb[:, :, xoff : xoff + xch],
                    xT_r[:, :, xoff : xoff + xch],
                )
                xoff += xch

            # Round-robin chunk emission over the 4 channel tiles: each c's
            # scan chain advances every round, all four drain concurrently at
            # the end (tail = ONE chunk of DVE work, not a whole c's worth),
            # and the first round only needs the first 1024 x-columns.
            # Round 0 is emitted in two phases (all F/I matmuls + sigmoids
            # for the 4 c-tiles first, then the H paths + DVE chains): during
            # pipeline ramp the DVE queue head is fp_fused, which needs only
            # sigf/sigi — phase-A emission feeds it 2x sooner.
            if L >= 4096:
                per_c = [512] + [1024] * ((L - 1024) // 1024) + [512]
                clists = {c: list(per_c) for c in range(NCT)}
                order = [(c, 0, "A") for c in range(NCT)]
                order += [(c, 0, "B") for c in range(NCT)]
                order += [
                    (c, j, "AB")
                    for j in range(1, len(per_c))
                    for c in range(NCT)
                ]
            else:
                clists = {c: [512] * (L // 512) for c in range(NCT)}
                order = [(c, j, "AB") for c in range(NCT)
                         for j in range(len(clists[c]))]
            hvs = {}
            lsoff = {c: 0 for c in range(NCT)}
            fi_stash = {}
            for c, lh, ph in order:
                if lh == 0 and c not in hvs:
                    hvs[c] = scan_pool.tile([P, L], F16, tag="hv", name=f"hv{c}")
                hv = hvs[c]
                LHC = clists[c][lh]
                ls = lsoff[c]
                if ph == "B":
                    sigf, sigi = fi_stash.pop(c)
                else:
                    sigf = sig3_pool.tile([P, LHC], F16, tag="sigf")
                    sigi = sig3_pool.tile([P, LHC], F16, tag="sigi")
                if ph != "A":
                    shlh = gates_pool.tile([P, LHC], F16, tag="shlh")
                    rl = gates_pool.tile([P, LHC], F16, tag="rl")
                if True:

                    # One 4-bank PSUM tile per gate; each gate's sigmoid is a
                    # single full-chunk ACT instruction (less ACT overhead).
                    def gate_mms8(ps, ocol):
                        # fp8 DoubleRow: each matmul contracts TWO 128-row
                        # k-blocks (lhsT free dims [2, P], rhs [2, LT]).
                        # kp-outer so consecutive matmuls share the stationary
                        # weights (amortizes the exposed 107ns LDWEIGHTS).
                        for kp in range(NK // 2):
                            for j in range(LHC // LT):
                                xk = slice(ls + j * LT, ls + (j + 1) * LT)
                                jl = slice(j * LT, (j + 1) * LT)
                                nc.tensor.matmul(
                                    ps[:, jl],
                                    wT8_sb[:, 2 * kp : 2 * kp + 2, ocol : ocol + P],
                                    xT8_sb[:, 2 * kp : 2 * kp + 2, xk],
                                    start=kp == 0,
                                    stop=kp == NK // 2 - 1,
                                    perf_mode=DoubleRow,
                                )

                    def gate_mms16(ps, ocol):
                        for j in range(LHC // LT):
                            xk = slice(ls + j * LT, ls + (j + 1) * LT)
                            jl = slice(j * LT, (j + 1) * LT)
                            for ki in range(NK):
                                nc.tensor.matmul(
                                    ps[:, jl],
                                    wT_sb[:, ki, ocol : ocol + P],
                                    xT_sb[:, ki, xk],
                                    start=ki == 0,
                                    stop=ki == NK - 1,
                                )

                    def do_F():
                        ps = psum_pool.tile([P, LHC], F32, tag="ps")
                        gate_mms8(ps, (c * 2 + 0) * P)
                        nc.scalar.activation(
                            sigf[:], ps[:], Act.Sigmoid,
                            bias=bias_sb[:, 0 * NCT + c : 0 * NCT + c + 1],
                        )

                    def do_I():
                        ps = psum_pool.tile([P, LHC], F32, tag="ps")
                        gate_mms8(ps, (c * 2 + 1) * P)
                        nc.scalar.activation(
                            sigi[:], ps[:], Act.Sigmoid,
                            bias=bias_sb[:, 1 * NCT + c : 1 * NCT + c + 1],
                        )

                    def do_H():
                        ps = psum_pool.tile([P, LHC], F32, tag="ps")
                        gate_mms16(ps, c * P)
                        nc.scalar.activation(
                            shlh[:], ps[:], Act.Sigmoid,
                            bias=bias_sb[:, 2 * NCT + c : 2 * NCT + c + 1],
                        )
                        return ps

                    if ph != "B":
                        do_F()
                        do_I()
                    if ph == "A":
                        fi_stash[c] = (sigf, sigi)
                        continue
                    psH = do_H()
                    # DVE queue order matters (in-order engine): fp_fused and
                    # fpm1 depend only on the chunk's FIRST two ACT outputs,
                    # so they go ahead of the g computation.
                    fp = gates_pool.tile([P, LHC], F16, tag="fp")
                    nc.vector._custom_dve(
                        FP_OP, out=fp[:], in0=sigf[:], in1=sigi[:],
                        s0=RECIP_C0, s1=RECIP_C1,
                    )
                    # fpm1 = fp - 1 stays on DVE: shifting it to ACT balances
                    # engine totals but lengthens the per-chunk latency chain
                    # (cross-engine hop) and measures ~2us slower.
                    fpm1 = gates_pool.tile([P, LHC], F16, tag="fpm1")
                    nc.vector.tensor_scalar_add(fpm1[:], fp[:], -1.0)
                    # g = max(x + 0.5, sigmoid(x)), x = psH + b_h (exact
                    # identity): rl = x + b + 0.5 on ACT, then a 2x fp16 DVE
                    # max. (A fused 1x HTIL_MAX custom on DVE was tried and
                    # is a net loss: the DVE is the saturated engine.)
                    nc.scalar.activation(
                        rl[:], psH[:], Act.Identity,
                        bias=bias_sb[:, 3 * NCT + c : 3 * NCT + c + 1],
                    )
                    nc.vector.tensor_tensor(rl[:], rl[:], shlh[:], Alu.max)
                    wv = gates_pool.tile([P, LHC], F16, tag="wv")
                    nc.vector.tensor_tensor(wv[:], fpm1[:], rl[:], Alu.mult)
                    init = HX_INIT if lh == 0 else hv[:, ls - 1 : ls]
                    nc.vector.tensor_tensor_scan(
                        hv[:, ls : ls + LHC], fp[:], wv[:], init,
                        Alu.mult, Alu.subtract,
                    )
                    nc.sync.dma_start(
                        out_d[c * P : (c + 1) * P, ls : ls + LHC],
                        hv[:, ls : ls + LHC],
                    )
                    lsoff[c] += LHC

    nc.compile()
    _cached_nc[key] = nc
    return nc


def prep_in_maps(x: np.ndarray, W: np.ndarray, b: np.ndarray):
    import ml_dtypes

    e4m3 = ml_dtypes.float8_e4m3fn
    W = np.asarray(W)
    # F/I rows grouped per c-tile for the fp8 path; H rows for the fp16 path
    idx_fi = np.concatenate(
        [np.arange(g * H + c * P, g * H + (c + 1) * P) for c in range(NCT) for g in range(2)]
    )
    idx_h = np.arange(2 * H, 3 * H)
    wT8 = np.ascontiguousarray(W[idx_fi].T).astype(e4m3)
    wT16 = np.ascontiguousarray(W[idx_h].T).astype(np.float16)
    b32 = np.asarray(b, dtype=np.float32)
    bias = np.empty((P, 16), dtype=np.float32)
    for j in range(12):
        bias[:, j] = b32[j * P : (j + 1) * P]
    for c in range(NCT):
        bias[:, 12 + c] = b32[2 * H + c * P : 2 * H + (c + 1) * P] + 0.5
    return [
        {
            "xT8": np.ascontiguousarray(np.asarray(x[n]).T).astype(e4m3),
            "xT": np.ascontiguousarray(np.asarray(x[n]).T).astype(np.float16),
            "wT8": wT8,
            "wT": wT16,
            "bias": bias,
        }
        for n in range(N)
    ]


def _run(x, W, b, **spmd_kw):
    from concourse.bass_utils import run_bass_kernel_spmd

    nc = build_program()
    in_maps = prep_in_maps(x, W, b)
    res = run_bass_kernel_spmd(nc, in_maps, list(range(N)), **spmd_kw)
    out = np.empty((N, L, H), dtype=np.float32)
    for n in range(N):
        out[n] = res.results[n]["out"].T.astype(np.float32)
    return out, res


def kernel(x: np.ndarray, W: np.ndarray, b: np.ndarray) -> np.ndarray:
    out, _ = _run(x, W, b)
    return out



# revision 68
# speedup vs baseline: 1.0757x; 1.0757x over previous
"""MinLSTM cell (Heinsen-scan reference) as a Bass/Tile kernel for 8 trn2 NeuronCores.

The reference's log-space parallel scan is rewritten exactly in linear space:
    h_t = f'_t h_{t-1} + (1 - f'_t) g(pre_h_t),   h_0 = 1e-6
with f' = sigmoid(pre_f+b_f) / (sigmoid(pre_f+b_f) + sigmoid(pre_i+b_i)) and
g(x) = x>=0 ? x+0.5 : sigmoid(x). This is a convex combination of positive
terms, so it is numerically stable without log-space.

Distribution: data-parallel over batch N=8, one batch element per core, W/b
replicated. Host-side prep transposes x[n] to [H_in, L] and ships it TWICE:
e4m3 for the F/I gate matmuls (DoubleRow perf mode = 2 k-blocks per matmul
at ~2x PE rate; the f/(f+i) normalization damps fp8 error to ~8e-3) and fp16
for the H gate (its error hits the output with slope 1 — fp8 there alone
costs 1.8e-2 rel err, over the gate). W rows are regrouped per 128-channel
c-tile. Device output is [H, L] fp16, transposed/upcast on the host.

Per-core device pipeline (chunks round-robin over the 4 c-tiles, sizes
[512, 1024 x3, 512] per c so all four scan chains start early and drain
concurrently at the end):
  PE : psF/psI via fp8 DoubleRow matmuls; psH via fp16 matmuls (fp32 acc)
  ACT: sigf = sigmoid(psF+b_f); sigi = sigmoid(psI+b_i)   (fp16 out)
       shlh = sigmoid(psH+b_h); rl = psH + (b_h+0.5)      (Identity w/ bias)
  DVE: fp   = FP_FUSED_MINLSTM(sigf, sigi)   custom op: sf * recip1nr(sf+si)
              (bit-trick recip seed + one NR, 7 uop stages, ~0.2% max err)
       fpm1 = fp - 1                         (tensor_scalar, 4x fp16)
       htil = max(rl, shlh)  == g, exact     (tensor_tensor, 2x fp16)
       wv   = fpm1 * htil                    (tensor_tensor, 2x fp16)
       h    = tensor_tensor_scan(d0=fp, d1=wv, op0=mult, op1=subtract)
              => h_t = f'_t h_{t-1} - wv_t, chained across chunks via initial=
  DMA: h chunk -> HBM (sync queue; x16 also sync, x8 scalar — HWDGE only)
The DVE is the bottleneck engine (~84us busy: the 2 cyc/elem scan plus the
1x custom op; the rest run 2x/4x); PE ~66us, ACT ~74us, all overlapped by
the Tile scheduler. Startup floor is ~12us: engine preamble + the one-time
sigmoid act-table load gate the first real ACT op.
"""

import os
import sys

import numpy as np

sys.path.insert(0, "/opt/trn_rl_repo")

import concourse.bass as bass  # noqa: E402
import concourse.tile as tile  # noqa: E402
from concourse import bacc, mybir  # noqa: E402
import concourse.dve_ops as dve_ops_mod  # noqa: E402
from concourse.dve_spec import (  # noqa: E402
    C0,
    C1,
    AluOp,
    Bin,
    Spec,
    Src0,
    Src1,
    maxx,
)

RECIP_C0 = -0.23549792
RECIP_C1 = 2.0017324


def _make_dve_ops():
    """Register two kernel-specific fused DVE ops (idempotent).

    FP_FUSED_MINLSTM: out = Src0 * recip_1nr(Src0 + Src1) — the gate
    normalization f' = sf/(sf+si) in ONE 7-stage instruction (bit-trick
    reciprocal seed + one NR pass, ~0.2% max err; a second NR would not fit
    the 8-stage budget). Kills the CCE accumulate, the separate recip op and
    the f' multiply, and lets sigi be fp16.

    HTIL_MAX_MINLSTM: out = max(Src0 + C0, Src1) — g(x) = max(x+b+0.5,
    sigmoid(x+b)) with the per-partition bias riding C0, reading pre_h
    straight from PSUM. Kills the ACT Identity pass.
    """
    if "FP_FUSED_MINLSTM" in dve_ops_mod._SUB_OPCODE_FOR_NAME:
        by_name = {op.name: op for op in dve_ops_mod.OPS}
        return by_name["FP_FUSED_MINLSTM"], by_name["HTIL_MAX_MINLSTM"]

    _s = Bin(AluOp.ADD, Src0, Src1)
    _ns = Bin(AluOp.BITWISE_NOT, _s, _s)
    _y0 = _ns * C0
    _y1 = _y0 * (C1 - _s * _y0)

    def _ref_fp(in0, in1, c0, c1, c2):
        s = in0.astype(np.float32) + in1.astype(np.float32)
        not_x = (~s.view(np.int32)).view(np.float32)
        y0 = not_x * c0
        y1 = y0 * (c1 - s * y0)
        return y1 * in0.astype(np.float32)

    fp_op = dve_ops_mod.DveOp(
        "FP_FUSED_MINLSTM",
        Spec(body=_y1 * Src0, reference=_ref_fp),
        subdim=False,
        uops_sha={},
    )

    def _ref_htil(in0, in1, c0, c1, c2):
        return np.maximum(in0.astype(np.float32) + c0, in1.astype(np.float32))

    htil_op = dve_ops_mod.DveOp(
        "HTIL_MAX_MINLSTM",
        Spec(body=maxx(Src0 + C0, Src1), reference=_ref_htil),
        subdim=False,
        uops_sha={},
    )

    import re

    for op in (fp_op, htil_op):
        dve_ops_mod.OPS.append(op)
        dve_ops_mod._SUB_OPCODE_FOR_NAME[op.name] = (
            dve_ops_mod._CUSTOM_DVE_ROW_BASE + len(dve_ops_mod.OPS) - 1
        )
        dve_ops_mod.CUSTOM_DVE_SPECS[op.name] = op.spec
        for ver in ("v3", "v4"):
            try:
                op.compile(ver)
            except ValueError as e:
                m = re.search(r'="([0-9a-f]+)"', str(e))
                if not m:
                    raise
                op.uops_sha[ver] = m.group(1)
                op.compile(ver)
    return fp_op, htil_op

N, L, H_IN, H = 8, 4096, 512, 512
H3 = 3 * H
P = 128
NK = H_IN // P  # 4 k-blocks of the contraction dim
NCT = H // P  # 4 channel tiles
LT = 512  # psum/matmul l-tile (one PSUM bank of fp32)
LH = 2048  # l-granularity of the big DVE ops
NLT = L // LT
NLH = L // LH

F32 = mybir.dt.float32
F16 = mybir.dt.float16
F8 = mybir.dt.float8e4
Alu = mybir.AluOpType
Act = mybir.ActivationFunctionType
DoubleRow = mybir.MatmulPerfMode.DoubleRow

HX_INIT = 1e-6

_cached_nc = {}


def build_program(L=L, LH=LH):
    key = (L, LH)
    if key in _cached_nc:
        return _cached_nc[key]
    NLH = L // LH
    FP_OP, HTIL_OP = _make_dve_ops()

    nc = bacc.Bacc()
    # F/I gates run e4m3 DoubleRow matmuls (2x PE rate, gate error is damped
    # by the f/(f+i) normalization); the H gate runs fp16 (its error hits the
    # output with slope ~1, fp8 there alone costs 1.8e-2 rel err).
    xT8_d = nc.dram_tensor("xT8", [H_IN, L], F8, kind="ExternalInput")
    xT_d = nc.dram_tensor("xT", [H_IN, L], F16, kind="ExternalInput")
    wT8_d = nc.dram_tensor("wT8", [H_IN, 2 * H], F8, kind="ExternalInput")
    wT_d = nc.dram_tensor("wT", [H_IN, H], F16, kind="ExternalInput")
    bias_d = nc.dram_tensor("bias", [P, 16], F32, kind="ExternalInput")
    out_d = nc.dram_tensor("out", [H, L], F16, kind="ExternalOutput")

    with tile.TileContext(nc) as tc:
        with (
            tc.tile_pool(name="const", bufs=1) as const_pool,
            tc.tile_pool(name="gates", bufs=3) as gates_pool,
            tc.tile_pool(name="sig3", bufs=6) as sig3_pool,
            tc.tile_pool(name="scanbuf", bufs=4) as scan_pool,
            tc.tile_pool(name="psum", bufs=4, space="PSUM") as psum_pool,
        ):
            # Warmup activation with minimal sync deps: absorbs the one-time
            # sigmoid act-table load (walrus rejects table-load + multi-wait
            # on one Activation instruction).
            warm = const_pool.tile([P, 8], F32)
            nc.vector.memset(warm[:], 0.0)
            neg1 = const_pool.tile([P, 1], F32)
            nc.vector.memset(neg1[:], -1.0)
            # bias= so the instruction form matches the real gate sigmoids
            # (a formally different activation triggers a second 1.3us
            # LoadActFuncSet at startup).
            nc.scalar.activation(
                warm[:], warm[:], Act.Sigmoid, bias=warm[:, 0:1]
            )
            # PE warmup: ~2us of garbage matmuls with no dependencies, so the
            # HAM clock gate ramps while the first DMAs are in flight. (Kept
            # short: an in-order PE queue means every extra warmup matmul
            # delays the first real one.)
            wup = const_pool.tile([P, P], F16)
            nc.vector.memset(wup[:], 0.0)
            wup_ps = psum_pool.tile([P, P], F32, tag="ps")
            # Enough garbage matmuls to keep the PE clock from dropping back
            # to the LOW pstate while the first W/x DMAs land (~2.3us idle
            # otherwise; idle early matmuls run 4x slow at 0.65GHz).
            for _ in range(12):
                nc.tensor.matmul(wup_ps[:], wup[:], wup[:], start=True, stop=True)

            xT8_sb = const_pool.tile([P, NK, L], F8)
            xT_sb = const_pool.tile([P, NK, L], F16)
            wT8_sb = const_pool.tile([P, NK, 2 * H], F8)
            wT_sb = const_pool.tile([P, NK, H], F16)
            bias_sb = const_pool.tile([P, 16], F32)

            # W columns are host-reordered grouped by c-tile: wT8 holds each
            # c-tile's F/I blocks contiguous, wT (fp16) the H blocks; the
            # first DMA of each unblocks c=0. bias rides the sync queue.
            wT8_r = wT8_d.rearrange("(ki p) o -> p ki o", p=P)
            wT_r = wT_d.rearrange("(ki p) o -> p ki o", p=P)
            xT8_r = xT8_d.rearrange("(ki p) l -> p ki l", p=P)
            xT_r = xT_d.rearrange("(ki p) l -> p ki l", p=P)
            if L >= 4096:
                xchunks = [512] + [2048] * ((L - 2048) // 2048) + [1536]
            else:
                xchunks = [512] * (L // 512)
            CW = 2 * P

            # Sync-queue order: c0 weights + bias, FIRST x16 chunk (so round
            # 0 unblocks asap), then the remaining weight groups, then the
            # rest of x16. x8 rides the scalar queue (both HWDGE; the gpsimd
            # SWDGE queue generates descriptors in software — far too slow).
            def w_group(cg):
                nc.sync.dma_start(
                    wT8_sb[:, :, cg * CW : (cg + 1) * CW],
                    wT8_r[:, :, cg * CW : (cg + 1) * CW],
                )
                nc.sync.dma_start(
                    wT_sb[:, :, cg * P : (cg + 1) * P],
                    wT_r[:, :, cg * P : (cg + 1) * P],
                )

            # Head DMAs all on the sync queue; they land serially (~1.3-2us
            # each), and the critical path is psF <- F-matmuls <- {W8c0,
            # x8#1}, so those two lead. Alternatives measured WORSE: scalar
            # queue stalls the activation stream (+19us); gpsimd SWDGE has a
            # ~8us descriptor stall (+4us).
            nc.sync.dma_start(
                wT8_sb[:, :, :CW], wT8_r[:, :, :CW]
            )
            nc.sync.dma_start(
                xT8_sb[:, :, : xchunks[0]], xT8_r[:, :, : xchunks[0]]
            )
            nc.sync.dma_start(bias_sb[:], bias_d[:])
            nc.sync.dma_start(
                xT_sb[:, :, : xchunks[0]], xT_r[:, :, : xchunks[0]]
            )
            nc.sync.dma_start(
                wT_sb[:, :, :P], wT_r[:, :, :P]
            )
            for cg in range(1, NCT):
                w_group(cg)
            xoff = xchunks[0]
            for xch in xchunks[1:]:
                nc.sync.dma_start(
                    xT8_sb[:, :, xoff : xoff + xch],
                    xT8_r[:, :, xoff : xoff + xch],
                )
                nc.sync.dma_start(
                    xT_sb[:, :, xoff : xoff + xch],
                    xT_r[:, :, xoff : xoff + xch],
                )
                xoff += xch

            # Round-robin chunk emission over the 4 channel tiles: each c's
            # scan chain advances every round, all four drain concurrently at
            # the end (tail = ONE chunk of DVE work, not a whole c's worth),
            # and the first round only needs the first 1024 x-columns.
            if L >= 4096:
                per_c = [512] + [1024] * ((L - 1024) // 1024) + [512]
                clists = {c: list(per_c) for c in range(NCT)}
                order = [
                    (c, j) for j in range(len(per_c)) for c in range(NCT)
                ]
            else:
                clists = {c: [512] * (L // 512) for c in range(NCT)}
                order = [(c, j) for c in range(NCT)
                         for j in range(len(clists[c]))]
            hvs = {}
            lsoff = {c: 0 for c in range(NCT)}
            for c, lh in order:
                if lh == 0:
                    hvs[c] = scan_pool.tile([P, L], F16, tag="hv", name=f"hv{c}")
                hv = hvs[c]
                LHC = clists[c][lh]
                ls = lsoff[c]
                if True:
                    sigf = sig3_pool.tile([P, LHC], F16, tag="sigf")
                    sigi = sig3_pool.tile([P, LHC], F16, tag="sigi")
                    shlh = gates_pool.tile([P, LHC], F16, tag="shlh")
                    rl = gates_pool.tile([P, LHC], F16, tag="rl")

                    # One 4-bank PSUM tile per gate; each gate's sigmoid is a
                    # single full-chunk ACT instruction (less ACT overhead).
                    def gate_mms8(ps, ocol):
                        # fp8 DoubleRow: each matmul contracts TWO 128-row
                        # k-blocks (lhsT free dims [2, P], rhs [2, LT]).
                        # kp-outer so consecutive matmuls share the stationary
                        # weights (amortizes the exposed 107ns LDWEIGHTS).
                        for kp in range(NK // 2):
                            for j in range(LHC // LT):
                                xk = slice(ls + j * LT, ls + (j + 1) * LT)
                                jl = slice(j * LT, (j + 1) * LT)
                                nc.tensor.matmul(
                                    ps[:, jl],
                                    wT8_sb[:, 2 * kp : 2 * kp + 2, ocol : ocol + P],
                                    xT8_sb[:, 2 * kp : 2 * kp + 2, xk],
                                    start=kp == 0,
                                    stop=kp == NK // 2 - 1,
                                    perf_mode=DoubleRow,
                                )

                    def gate_mms16(ps, ocol):
                        for j in range(LHC // LT):
                            xk = slice(ls + j * LT, ls + (j + 1) * LT)
                            jl = slice(j * LT, (j + 1) * LT)
                            for ki in range(NK):
                                nc.tensor.matmul(
                                    ps[:, jl],
                                    wT_sb[:, ki, ocol : ocol + P],
                                    xT_sb[:, ki, xk],
                                    start=ki == 0,
                                    stop=ki == NK - 1,
                                )

                    def do_F():
                        ps = psum_pool.tile([P, LHC], F32, tag="ps")
                        gate_mms8(ps, (c * 2 + 0) * P)
                        nc.scalar.activation(
                            sigf[:], ps[:], Act.Sigmoid,
                            bias=bias_sb[:, 0 * NCT + c : 0 * NCT + c + 1],
                        )

                    def do_I():
                        ps = psum_pool.tile([P, LHC], F32, tag="ps")
                        gate_mms8(ps, (c * 2 + 1) * P)
                        nc.scalar.activation(
                            sigi[:], ps[:], Act.Sigmoid,
                            bias=bias_sb[:, 1 * NCT + c : 1 * NCT + c + 1],
                        )

                    def do_H():
                        ps = psum_pool.tile([P, LHC], F32, tag="ps")
                        gate_mms16(ps, c * P)
                        nc.scalar.activation(
                            shlh[:], ps[:], Act.Sigmoid,
                            bias=bias_sb[:, 2 * NCT + c : 2 * NCT + c + 1],
                        )
                        return ps

                    do_F()
                    do_I()
                    psH = do_H()
                    # DVE queue order matters (in-order engine): fp_fused and
                    # fpm1 depend only on the chunk's FIRST two ACT outputs,
                    # so they go ahead of the g computation.
                    fp = gates_pool.tile([P, LHC], F16, tag="fp")
                    nc.vector._custom_dve(
                        FP_OP, out=fp[:], in0=sigf[:], in1=sigi[:],
                        s0=RECIP_C0, s1=RECIP_C1,
                    )
                    # fpm1 = fp - 1 stays on DVE: shifting it to ACT balances
                    # engine totals but lengthens the per-chunk latency chain
                    # (cross-engine hop) and measures ~2us slower.
                    fpm1 = gates_pool.tile([P, LHC], F16, tag="fpm1")
                    nc.vector.tensor_scalar_add(fpm1[:], fp[:], -1.0)
                    # g = max(x + 0.5, sigmoid(x)), x = psH + b_h (exact
                    # identity): rl = x + b + 0.5 on ACT, then a 2x fp16 DVE
                    # max. (A fused 1x HTIL_MAX custom on DVE was tried and
                    # is a net loss: the DVE is the saturated engine.)
                    nc.scalar.activation(
                        rl[:], psH[:], Act.Identity,
                        bias=bias_sb[:, 3 * NCT + c : 3 * NCT + c + 1],
                    )
                    nc.vector.tensor_tensor(rl[:], rl[:], shlh[:], Alu.max)
                    wv = gates_pool.tile([P, LHC], F16, tag="wv")
                    nc.vector.tensor_tensor(wv[:], fpm1[:], rl[:], Alu.mult)
                    init = HX_INIT if lh == 0 else hv[:, ls - 1 : ls]
                    nc.vector.tensor_tensor_scan(
                        hv[:, ls : ls + LHC], fp[:], wv[:], init,
                        Alu.mult, Alu.subtract,
                    )
                    nc.sync.dma_start(
                        out_d[c * P : (c + 1) * P, ls : ls + LHC],
                        hv[:, ls : ls + LHC],
                    )
                    lsoff[c] += LHC

    nc.compile()
    _cached_nc[key] = nc
    return nc


def prep_in_maps(x: np.ndarray, W: np.ndarray, b: np.ndarray):
    import ml_dtypes

    e4m3 = ml_dtypes.float8_e4m3fn
    W = np.asarray(W)
    # F/I rows grouped per c-tile for the fp8 path; H rows for the fp16 path
    idx_fi = np.concatenate(
        [np.arange(g * H + c * P, g * H + (c + 1) * P) for c in range(NCT) for g in range(2)]
    )
    idx_h = np.arange(2 * H, 3 * H)
    wT8 = np.ascontiguousarray(W[idx_fi].T).astype(e4m3)
    wT16 = np.ascontiguousarray(W[idx_h].T).astype(np.float16)
    b32 = np.asarray(b, dtype=np.float32)
    bias = np.empty((P, 16), dtype=np.float32)
    for j in range(12):
        bias[:, j] = b32[j * P : (j + 1) * P]
    for c in range(NCT):
        bias[:, 12 + c] = b32[2 * H + c * P : 2 * H + (c + 1) * P] + 0.5
    return [
        {
            "xT8": np.ascontiguousarray(np.asarray(x[n]).T).astype(e4m3),
            "xT": np.ascontiguousarray(np.asarray(x[n]).T).astype(np.float16),
            "wT8": wT8,
            "wT": wT16,
            "bias": bias,
        }
        for n in range(N)
    ]


def _run(x, W, b, **spmd_kw):
    from concourse.bass_utils import run_bass_kernel_spmd

    nc = build_program()
    in_maps = prep_in_maps(x, W, b)
    res = run_bass_kernel_spmd(nc, in_maps, list(range(N)), **spmd_kw)
    out = np.empty((N, L, H), dtype=np.float32)
    for n in range(N):
        out[n] = res.results[n]["out"].T.astype(np.float32)
    return out, res


def kernel(x: np.ndarray, W: np.ndarray, b: np.ndarray) -> np.ndarray:
    out, _ = _run(x, W, b)
    return out

